# revision 63
# baseline (speedup 1.0000x reference)
"""Trainium2 Bass kernel for nn_NodeAttDiff (segment-reduce node attention).

Math (reference):
    e1, e2 = out_gnn[:N], out_gnn[N:]          # N = 200000, D = 256
    diff   = e1 - e2
    h      = relu([e1 e2 diff] @ W1 + b1)      # folded: e1@WA + e2@WB, WA=W1a+W1c, WB=W1b-W1c
    raw    = (h @ W2 + b2)[:, 0]
    att    = segment_softmax(raw, batch)       # contiguous segments (batch sorted)
    out    = segment_sum(att[:,None] * diff)   # [512, 256]

Device strategy (8 cores, node-partitioned data parallel):
    - Exactly ceil(N/8) nodes per core (cuts are NOT graph aligned); each
      core emits raw per-graph sums [gw, 256+2] = [sum_n w_n diff_n | sum_n
      w_n | .] over its local graph window and the host adds the partial
      num/den of graphs split across cores, then divides:
          out_g = num_g / den_g,   w_n = exp(raw_n + b2)
      (softmax max-subtraction is skipped: raw is O(5), exp safe in fp32).
    - Host pre-transposes e1/e2 to feature-major fp16 (merged in one dram
      tensor, 1 DMA trigger per 2-tile group -- the Sync engine dispatches
      HW-DGE triggers at ~650ns each and is the startup bottleneck) and ALSO
      sends node-major [diff | 1,1] rows, so the device never transposes
      diff; the segment-sum rhs comes straight from DMA (triggered from the
      Scalar queue to spread trigger dispatch).
    - raw is computed with h as the *stationary* matmul operand:
        raw[128n, 2] += h_chunk[128f, 128n]^T @ w2_chunk[128f, 2]
      streaming only w2's 2 columns, so raw lands NODE-major; exp applies
      directly and the one-hot scale (sw) needs no transpose trick.
    - Per tile t of 512 nodes:
        z^T   = WA.T @ e1T + WB.T @ e2T                  (8 matmuls, PSUM)
        h^T   = relu(z^T + b1)                           (ACT, PSUM->SBUF)
        raw   = h^T(stationary) @ w2   [tile t-1]        (8 cheap matmuls)
        ew    = exp(raw + b2)          [tile t-1]        (ACT, [128,4])
        sw    = (iota == seg_id) * ew  [tile t-1]        (DVE tensor_scalar x4)
        seg  += sw.T @ [diff | 1]      [tile t-3]        (PSUM accumulate)
      The deferrals give the cross-engine chains slack so the MM pipe never
      waits on ACT/DVE.
"""

import os
import ml_dtypes
import numpy as np

NUM_GRAPHS = 512
N_CORES = 8
D = 256
TILE_N = 512  # nodes per tile


_CACHE = {}


def _build_program(cap: int, gw: int):
    """Build + compile the SPMD Bass program; `cap` nodes and a `gw`-graph
    window per core."""
    if (cap, gw) in _CACHE:
        return _CACHE[(cap, gw)]

    from contextlib import ExitStack
    import concourse.bass as bass
    import concourse.tile as tile
    import concourse.bacc as bacc
    import concourse.mybir as mybir

    f32 = mybir.dt.float32
    f16 = mybir.dt.float16
    AF = mybir.ActivationFunctionType
    ALU = mybir.AluOpType

    assert cap % TILE_N == 0
    n_tiles = cap // TILE_N
    n_groups = (n_tiles + 1) // 2
    n_cols = cap // 128  # bm columns

    nc = bacc.Bacc("TRN2", target_bir_lowering=False, debug=False,
                   num_devices=N_CORES)

    e1t_d = nc.dram_tensor("e1t", [2, 128, cap], f16, kind="ExternalInput").ap()
    e2t_d = nc.dram_tensor("e2t", [2, 128, cap], f16, kind="ExternalInput").ap()
    dfn_d = nc.dram_tensor("dfn", [128, n_cols, D + 2], f16,
                           kind="ExternalInput").ap()
    # ALL consts AND the first e-group packed into one f16 dram tensor (plus
    # one f32): each dma_start's completion semaphore serializes ~1-1.5us
    # behind the previous one at startup, so the first z matmul wants ONE
    # preceding DMA.  c16 = [wa(512) | wb(512) | w2(4) | e12 group0 (4096)];
    # wa cols (k*2+m)*128+n hold WA[k*128+p, m*128+n]; e12 group-0 cols
    # (si*2+k)*1024 + n hold e{si+1}T[k*128+p, n] for n in [0, 1024).
    # c32 = [b1(2) | b2(1) | iota(gw) | bm(n_cols)].
    c16_d = nc.dram_tensor("c16", [128, 1028 + 4 * 2 * TILE_N], f16,
                           kind="ExternalInput").ap()
    c32_d = nc.dram_tensor("c32", [128, 3 + gw + n_cols], f32,
                           kind="ExternalInput").ap()
    # group 1 in its own single packed DMA (one completion semaphore instead
    # of two) so tiles 2-3 never stall; same col layout as the g0 block
    g1p_d = nc.dram_tensor("g1p", [128, 4 * 2 * TILE_N], f16,
                           kind="ExternalInput").ap()
    out_d = nc.dram_tensor("out", [gw, D + 2], f32, kind="ExternalOutput").ap()

    with tile.TileContext(nc) as tc:
        with ExitStack() as ctx:
            consts = ctx.enter_context(tc.tile_pool(name="consts", bufs=1))
            epool = ctx.enter_context(tc.tile_pool(name="epool", bufs=6))
            dpool = ctx.enter_context(tc.tile_pool(name="dpool", bufs=4))
            hpool = ctx.enter_context(tc.tile_pool(name="hpool", bufs=4))
            spool = ctx.enter_context(tc.tile_pool(name="spool", bufs=6))
            zpool = ctx.enter_context(
                tc.tile_pool(name="zpool", bufs=5, space=bass.MemorySpace.PSUM))
            rawpool = ctx.enter_context(
                tc.tile_pool(name="rawpool", bufs=2, space=bass.MemorySpace.PSUM))
            segpool = ctx.enter_context(
                tc.tile_pool(name="segpool", bufs=1, space=bass.MemorySpace.PSUM))

            # ---- constants (2 packed DMAs; views are slices of the packs) --
            c16 = consts.tile([128, 1028 + 4 * 2 * TILE_N], f16, tag="c16")
            c32 = consts.tile([128, 3 + gw + n_cols], f32, tag="c32")
            g1p = consts.tile([128, 4 * 2 * TILE_N], f16, tag="g1p")
            nc.sync.dma_start(c16[:], c16_d[:])
            nc.sync.dma_start(g1p[:], g1p_d[:])
            nc.sync.dma_start(c32[:], c32_d[:])

            def wmat_v(wi, k, m):  # wi 0=wa, 1=wb -> [128, 128]
                o = wi * 512 + (k * 2 + m) * 128
                return c16[:, o:o + 128]

            def w2_v(k):  # [128, 2]
                return c16[:, 1024 + 2 * k:1024 + 2 * k + 2]

            def b1_v(m):  # [128, 1]
                return c32[:, m:m + 1]

            b2_v = c32[:, 2:3]
            iota_v = c32[:, 3:3 + gw]

            def bm_v(col):  # [128, 1]
                return c32[:, 3 + gw + col:3 + gw + col + 1]

            def e12g01_v(g, si, k, nsl):  # [128, nodes] view into a pack
                if g == 0:
                    o = 1028 + (si * 2 + k) * (2 * TILE_N)
                    return c16[:, o + nsl.start:o + nsl.stop]
                o = (si * 2 + k) * (2 * TILE_N)
                return g1p[:, o + nsl.start:o + nsl.stop]

            def dfo_load(g, eng=None):
                tcount = min(2, n_tiles - 2 * g)
                dfo = dpool.tile([128, 4 * tcount, D + 2], f16,
                                 tag="dfo" if tcount == 2 else "dfol")
                (eng or nc.scalar).dma_start(
                    dfo[:], dfn_d[:, 8 * g:8 * g + 4 * tcount, :])
                return dfo

            # seg rhs layout: [diff(256) | ones(2)] -> out cols 0:256 sums,
            # 256:258 exp-sums
            seg = segpool.tile([gw, D + 2], f32, tag="seg")

            def e12_load(g):
                tcount = min(2, n_tiles - 2 * g)
                span = tcount * TILE_N
                e12 = [None, None]  # per-source tiles (v3-style granularity)
                for si, src in enumerate((e1t_d, e2t_d)):
                    e12[si] = epool.tile([128, 2, span], f16,
                                         tag=f"e{si}" if tcount == 2
                                         else f"e{si}l",
                                         name=f"e{si}_{g}")
                    dsl = slice(2 * g * TILE_N, 2 * g * TILE_N + span)
                    nc.sync.dma_start(
                        e12[si][:],
                        src[:, :, dsl].rearrange("k p n -> p k n"))
                return e12

            # first dfo from the Sync queue AFTER the startup packs so its
            # descriptors don't queue ahead of g1p's
            dfos = {0: dfo_load(0, eng=nc.sync)}  # group -> dfo tile

            # prefetch the next groups before any compute is queued (groups
            # 0 and 1 ride in the startup packs)
            epre = {}
            for gp in (2, 3):
                if gp < n_groups:
                    epre[gp] = e12_load(gp)
            state = {}  # tile idx -> per-tile tiles for deferred stages
            for g in range(n_groups):
                tcount = min(2, n_tiles - 2 * g)
                if g <= 1:
                    e12 = None
                else:
                    e12 = epre.pop(g)
                if g >= 1 and g + 4 < n_groups:
                    epre[g + 4] = e12_load(g + 4)
                elif g == 0 and 4 < n_groups:
                    epre[4] = e12_load(4)

                for ti in range(tcount):
                    t = 2 * g + ti
                    nsl = slice(ti * TILE_N, (ti + 1) * TILE_N)

                    def rhs_v(si, k):
                        if e12 is None:
                            return e12g01_v(g, si, k, nsl)
                        return e12[si][:, k, nsl]

                    # z^T [128, 512] per fo-chunk m
                    zc = [None, None]
                    for m in range(2):
                        zc[m] = zpool.tile([128, TILE_N], f32, tag="zr",
                                           name=f"z_{t}_{m}")
                        for wi in range(4):
                            k = wi % 2
                            nc.tensor.matmul(
                                zc[m][:], wmat_v(wi // 2, k, m),
                                rhs_v(wi // 2, k),
                                start=(wi == 0), stop=(wi == 3))

                    # h^T = relu(z + b1)  (ACT, PSUM -> SBUF)
                    h = hpool.tile([128, 2, TILE_N], f16, tag="h")
                    for m in range(2):
                        nc.scalar.activation(h[:, m, :], zc[m][:], AF.Relu,
                                             bias=b1_v(m), scale=1.0)
                    state[t] = {"h": h, "dfo": dfos[g], "ti": ti}

                    # deferred by 3 tiles: segment accumulate for tile t-3
                    # (before raw so the z(t+1) boundary follows the cheap
                    # raw matmuls, not the seg stationary switch)
                    if t - 3 in state:
                        st = state.pop(t - 3)
                        bo = 4 * st["ti"]
                        for b in range(4):
                            nc.tensor.matmul(
                                seg[:], st["sw"][:, b, :],
                                st["dfo"][:, bo + b, :],
                                start=(t - 3 == 0 and b == 0),
                                stop=False, skip_group_check=True)

                    # deferred by 1 tile: raw/exp/sw for tile t-1
                    if t - 1 in state:
                        st = state[t - 1]
                        hp = st["h"]
                        rawp = rawpool.tile([128, 4, 2], f32, tag="raw")
                        for b in range(4):
                            bsl = bass.ts(b, 128)
                            for k in range(2):
                                nc.tensor.matmul(
                                    rawp[:, b, :], hp[:, k, bsl], w2_v(k),
                                    start=(k == 0), stop=(k == 1),
                                    skip_group_check=True)
                        ew = spool.tile([128, 4, 1], f32, tag="ew")
                        nc.scalar.activation(ew[:], rawp[:, :, 0:1], AF.Exp,
                                             bias=b2_v, scale=1.0)
                        sw = spool.tile([128, 4, gw], f16, tag="sw")
                        for b in range(4):
                            nc.vector.tensor_scalar(
                                sw[:, b, :], iota_v,
                                bm_v(4 * (t - 1) + b),
                                ew[:, b, :], op0=ALU.is_equal, op1=ALU.mult)
                        st["sw"] = sw

                # prefetch next group's [diff | 1] rows from the ACT queue
                # (after this group's relu/exp so they aren't delayed)
                if g + 1 < n_groups:
                    dfos[g + 1] = dfo_load(g + 1)

            # drain: raw/exp/sw for the last tile, then remaining seg tiles
            st = state[n_tiles - 1]
            hp = st["h"]
            rawp = rawpool.tile([128, 4, 2], f32, tag="raw")
            for b in range(4):
                bsl = bass.ts(b, 128)
                for k in range(2):
                    nc.tensor.matmul(rawp[:, b, :], hp[:, k, bsl], w2_v(k),
                                     start=(k == 0), stop=(k == 1),
                                     skip_group_check=True)
            ew = spool.tile([128, 4, 1], f32, tag="ew")
            nc.scalar.activation(ew[:], rawp[:, :, 0:1], AF.Exp,
                                 bias=b2_v, scale=1.0)
            sw = spool.tile([128, 4, gw], f16, tag="sw")
            for b in range(4):
                nc.vector.tensor_scalar(
                    sw[:, b, :], iota_v,
                    bm_v(4 * (n_tiles - 1) + b),
                    ew[:, b, :], op0=ALU.is_equal, op1=ALU.mult)
            st["sw"] = sw
            for tt in sorted(state):
                st = state.pop(tt)
                bo = 4 * st["ti"]
                for b in range(4):
                    nc.tensor.matmul(seg[:], st["sw"][:, b, :],
                                     st["dfo"][:, bo + b, :],
                                     start=(tt == 0 and b == 0),
                                     stop=(tt == n_tiles - 1 and b == 3),
                                     skip_group_check=True)

            # tail: raw num/den out; the host does the division and combines
            # graphs split across core boundaries
            ot = spool.tile([gw, D + 2], f32, tag="ot")
            nc.vector.tensor_copy(ot[:], seg[:])
            nc.sync.dma_start(out_d[:], ot[:])

    nc.compile()
    _CACHE[(cap, gw)] = nc
    return nc


def _prepare(out_gnn, batch_input, W1, b1, W2, b2):
    out_gnn = np.asarray(out_gnn, dtype=np.float32)
    batch = np.asarray(batch_input, dtype=np.int64)
    W1 = np.asarray(W1, dtype=np.float32)
    b1 = np.asarray(b1, dtype=np.float32)
    W2 = np.asarray(W2, dtype=np.float32)
    b2 = np.asarray(b2, dtype=np.float32)

    half = out_gnn.shape[0] // 2
    batch = batch[:half]
    e1_all, e2_all = out_gnn[:half], out_gnn[half:]

    # exact node-balanced cuts (not graph aligned; the host later sums the
    # partial num/den of graphs split across two cores)
    per = -(-half // N_CORES)
    nbounds = np.minimum(np.arange(N_CORES + 1) * per, half)
    g0s = [int(batch[min(int(nbounds[c]), half - 1)]) for c in range(N_CORES)]
    spans = [int(batch[int(nbounds[c + 1]) - 1]) - g0s[c] + 1
             if nbounds[c + 1] > nbounds[c] else 1 for c in range(N_CORES)]
    gw = int(max(2, ((max(spans) + 1) // 2) * 2))
    max_n = int((nbounds[1:] - nbounds[:-1]).max())
    cap = max(TILE_N, ((max_n + TILE_N - 1) // TILE_N) * TILE_N)

    nc = _build_program(cap, gw)

    # host-folded MLP weights (fp64 for exactness, then fp16)
    W1a = W1[0:D].astype(np.float64)
    W1b = W1[D:2 * D].astype(np.float64)
    W1c = W1[2 * D:3 * D].astype(np.float64)
    WA = (W1a + W1c).astype(np.float32)
    WB = (W1b - W1c).astype(np.float32)

    def chunk4(w):  # [256,256] -> [p, ki*mo*128] (partition-major)
        return np.ascontiguousarray(
            w.astype(np.float16).reshape(2, 128, 2, 128).transpose(1, 0, 2, 3)
        ).reshape(128, 512)

    w2p = np.concatenate(
        [W2.astype(np.float16).reshape(2, 128, 1),
         np.zeros((2, 128, 1), np.float16)],
        axis=2).transpose(1, 0, 2).reshape(128, 4)
    c16_w = np.concatenate([chunk4(WA), chunk4(WB), w2p], axis=1)
    b1p = b1.reshape(2, 128).T.astype(np.float32)  # [128, 2]
    b2p = np.broadcast_to(b2.reshape(1, 1), (128, 1)).astype(np.float32)
    iotap = np.broadcast_to(np.arange(gw, dtype=np.float32), (128, gw))
    common = {}

    in_maps = []
    for c in range(N_CORES):
        s, e = int(nbounds[c]), int(nbounds[c + 1])
        n_c = e - s
        e12 = np.zeros((2, 2, 128, cap), dtype=np.float16)
        e12[0, :, :, :n_c] = e1_all[s:e].astype(np.float16).T.reshape(2, 128, n_c)
        e12[1, :, :, :n_c] = e2_all[s:e].astype(np.float16).T.reshape(2, 128, n_c)
        # node-major [diff | 1, 1] rows: dfn[p, cnk, :] = row of node cnk*128+p
        dfn = np.zeros((cap, D + 2), dtype=np.float16)
        dfn[:n_c, :D] = (e1_all[s:e] - e2_all[s:e]).astype(np.float16)
        dfn[:n_c, D:] = 1.0
        dfn = np.ascontiguousarray(
            dfn.reshape(cap // 128, 128, D + 2).transpose(1, 0, 2))
        bmv = np.full(cap, 999.0, dtype=np.float32)
        bmv[:n_c] = (batch[s:e] - g0s[c]).astype(np.float32)
        bmp = bmv.reshape(cap // 128, 128).T  # [128, n_cols]
        c32 = np.ascontiguousarray(
            np.concatenate([b1p, b2p, iotap, bmp], axis=1, dtype=np.float32))
        # first two e-groups ride in startup packs (one DMA completion each)
        def gflat(g):
            blk = np.zeros((2, 2, 128, 2 * TILE_N), dtype=np.float16)
            av = max(0, min(2 * TILE_N, cap - g * 2 * TILE_N))
            if av:
                blk[:, :, :, :av] = e12[:, :, :, g * 2 * TILE_N:
                                        g * 2 * TILE_N + av]
            return blk.transpose(2, 0, 1, 3).reshape(128, -1)

        c16 = np.ascontiguousarray(
            np.concatenate([c16_w, gflat(0)], axis=1, dtype=np.float16))
        in_maps.append({"e1t": np.ascontiguousarray(e12[0]),
                        "e2t": np.ascontiguousarray(e12[1]),
                        "dfn": dfn, "c16": c16, "c32": c32,
                        "g1p": np.ascontiguousarray(gflat(1)),
                        **common})
    return nc, in_maps, nbounds, g0s, gw


def _enable_ldw_opt():
    """Re-enable the compiler's weight-load optimization (off by default in
    this container's flag set); harmless no-op if the flag isn't present."""
    try:
        from concourse.compiler_utils import get_compiler_flags, set_compiler_flags
        flags = [f.replace("--enable-ldw-opt=false", "--enable-ldw-opt=true")
                 for f in get_compiler_flags()]
        set_compiler_flags(flags)
    except Exception:
        pass


def kernel(out_gnn, batch_input, W1, b1, W2, b2):
    import concourse.bass_utils as bass_utils

    _enable_ldw_opt()
    nc, in_maps, nbounds, g0s, gw = _prepare(
        out_gnn, batch_input, W1, b1, W2, b2)

    trace_dir = os.environ.get("NODEATT_TRACE_DIR")
    kw = {}
    if trace_dir:
        kw = {"trace": True, "tmpdir": trace_dir}
    res = bass_utils.run_bass_kernel_spmd(
        nc, in_maps, core_ids=list(range(N_CORES)), **kw)
    if trace_dir:
        kernel.last_exec_time_ns = res.exec_time_ns
        kernel.last_results = res

    num = np.zeros((NUM_GRAPHS, D), dtype=np.float64)
    den = np.zeros(NUM_GRAPHS, dtype=np.float64)
    for c in range(N_CORES):
        if nbounds[c + 1] <= nbounds[c]:
            continue
        part = res.results[c]["out"]  # [gw, D+2] f32
        g0 = g0s[c]
        ge = min(g0 + gw, NUM_GRAPHS)
        num[g0:ge] += part[:ge - g0, :D]
        den[g0:ge] += part[:ge - g0, D]
    out = (num / np.maximum(den, 1e-30)[:, None]).astype(np.float32)
    return out
